# revision 17
# baseline (speedup 1.0000x reference)
"""Fused multi-head attention + residual + LayerNorm for 8 trn2 NeuronCores.

Returns (out, attn) matching the reference:
  out  [4, 2048, 768] f32
  attn [4, 12, 2048, 2048] f32  (softmax probabilities -- the memory-bound output)

Sharding: core c handles batch b = c//2 and query-row half qh = c%2 (1024 rows).
Everything is local per core: K/V projections are recomputed by both cores of a
batch pair, queries/masks/outputs are split by query rows, LayerNorm rows are
fully local.

Per-core device pipeline (all matmuls bf16, PSUM f32):
  1. qT = Wq^T @ Q^T, kT = Wk^T @ K^T  (head-dim-on-partitions layout)
     vnat = V @ Wv with a constant 1.0 column appended per head (65-wide lhsT
     tiles later yield the per-column masked-exp sums for free).
  2. Per head: scores in BOTH orientations via lhsT/rhs swap:
       natural  [q,k]: exp -> mask-mult+rowsum (one DVE op) -> scale by 1/sum
                 -> f32 attn output tile -> DRAM
       transposed [k,q]: exp -> mask-mult -> context matmul (accumulate over k)
     The 65th row of the context PSUM holds sum_k(masked exp) per q; its
     reciprocal row is broadcast across partitions with a PE outer product and
     used to normalize context^T.
  3. out = context @ Wo + resid(=Q+bo), then LayerNorm per row, DMA out.
"""

import numpy as np
import ml_dtypes

import concourse.bass as bass
import concourse.mybir as mybir
import concourse.tile as tile
from concourse.bass_utils import run_bass_kernel_spmd

BF16 = ml_dtypes.bfloat16
F32 = mybir.dt.float32
BF = mybir.dt.bfloat16

B, S, D = 4, 2048, 768
H, DK = 12, 64
NQ = S // 2          # query rows per core
DC = D // 128        # 6 chunks of the model dim
QC = NQ // 128       # 8 query-row chunks per core
KC = S // 128        # 16 key chunks
SCALE = 1.0 / np.sqrt(DK).astype(np.float32)
EPS = 1e-5


def _split_multi_waits(nc):
    """walrus in this build rejects >1 sync wait on non-EventSemaphore
    instructions; Tile's tail drain can carry several DMA-queue waits.
    Split extras into single-wait NOPs placed just before the offender."""
    n = 0
    for fn in nc.m.functions:
        for bb in fn.blocks:
            new = []
            for ins in bb.instructions:
                si = getattr(ins, "sync_info", None)
                if (
                    si is not None
                    and si.on_wait
                    and len(si.on_wait) > 1
                    and not isinstance(ins, mybir.InstEventSemaphore)
                ):
                    extra, si.on_wait = list(si.on_wait[1:]), si.on_wait[:1]
                    for w in extra:
                        nop = mybir.InstNoOp(
                            name=f"{ins.name}-ws{n}",
                            engine=ins.engine,
                            ins=[],
                            outs=[],
                            sync_info=mybir.SyncInfo(on_wait=[w], on_update=[]),
                        )
                        new.append(nop)
                        n += 1
                new.append(ins)
            bb.instructions[:] = new
    return n


def _build_nc(waitsplit=True):
    nc = bass.Bass()

    qt_d = nc.dram_tensor("qt", [D, NQ], BF, kind="ExternalInput")
    kt_d = nc.dram_tensor("kt", [D, S], BF, kind="ExternalInput")
    vt_d = nc.dram_tensor("vt", [D, S], BF, kind="ExternalInput")
    wq_d = nc.dram_tensor("wq", [D, D], BF, kind="ExternalInput")
    wk_d = nc.dram_tensor("wk", [D, D], BF, kind="ExternalInput")
    wv_d = nc.dram_tensor("wv", [D, D], BF, kind="ExternalInput")
    wo_d = nc.dram_tensor("wo", [D, D], BF, kind="ExternalInput")
    bqt_d = nc.dram_tensor("bqt", [128, DC], F32, kind="ExternalInput")
    bkt_d = nc.dram_tensor("bkt", [128, DC], F32, kind="ExternalInput")
    bv_d = nc.dram_tensor("bv_row", [1, D], F32, kind="ExternalInput")
    m01_d = nc.dram_tensor("m01", [NQ, S], BF, kind="ExternalInput")
    m01t_d = nc.dram_tensor("m01t", [S, NQ], BF, kind="ExternalInput")
    resid_d = nc.dram_tensor("resid", [NQ, D], F32, kind="ExternalInput")
    lng_d = nc.dram_tensor("lng_row", [1, D], F32, kind="ExternalInput")
    lnb_d = nc.dram_tensor("lnb_row", [1, D], F32, kind="ExternalInput")

    attn_o = nc.dram_tensor("attn_o", [H, NQ, S], F32, kind="ExternalOutput")
    out_o = nc.dram_tensor("out_o", [NQ, D], F32, kind="ExternalOutput")

    with tile.TileContext(nc) as tc:
        # ---------- persistent SBUF ----------
        with tc.tile_pool(name="persist", bufs=1) as persist:
            qts = persist.tile([128, DC, NQ], BF)        # qT  [hdk, nq]
            kts = persist.tile([128, DC, S], BF)         # kT  [hdk, s]
            vns = persist.tile([128, KC, H, DK + 1], BF) # v natural + ones col
            ctxs = persist.tile([128, DC, NQ], BF)       # context^T [hdv, nq]

            ones = persist.tile([1, DK], BF)
            nc.vector.memset(ones, 1.0)
            nc.vector.memset(vns[:, :, :, DK:DK + 1], 1.0)

            # ---------- phase P: projections ----------
            with (
                tc.tile_pool(name="pin", bufs=1) as pin,
                tc.tile_pool(name="pw", bufs=1) as pw,
                tc.tile_pool(name="ppsum", bufs=4, space="PSUM") as ppsum,
                tc.tile_pool(name="pbias", bufs=1) as pbias,
            ):
                qtin = pin.tile([128, DC, NQ], BF)
                ktin = pin.tile([128, DC, S], BF)
                vtin = pin.tile([128, DC, S], BF)
                wqs = pw.tile([128, DC, D], BF)
                wks = pw.tile([128, DC, D], BF)
                wvs = pw.tile([128, DC, D], BF)
                bqts = pbias.tile([128, DC], F32)
                bkts = pbias.tile([128, DC], F32)
                bvb = pbias.tile([128, D], F32)

                nc.sync.dma_start(out=qtin, in_=qt_d.rearrange("(c p) n -> p c n", p=128))
                nc.sync.dma_start(out=ktin, in_=kt_d.rearrange("(c p) n -> p c n", p=128))
                nc.sync.dma_start(out=vtin, in_=vt_d.rearrange("(c p) n -> p c n", p=128))
                nc.sync.dma_start(out=wqs, in_=wq_d.rearrange("(c p) n -> p c n", p=128))
                nc.sync.dma_start(out=wks, in_=wk_d.rearrange("(c p) n -> p c n", p=128))
                nc.sync.dma_start(out=wvs, in_=wv_d.rearrange("(c p) n -> p c n", p=128))
                nc.sync.dma_start(out=bqts, in_=bqt_d[:, :])
                nc.sync.dma_start(out=bkts, in_=bkt_d[:, :])
                nc.sync.dma_start(out=bvb, in_=bv_d[:, :].to_broadcast((128, D)))

                # qT / kT: out rows = hdk (6 chunks of 128), contraction over D
                for dst, src, w, bias, ncols in (
                    (qts, qtin, wqs, bqts, NQ),
                    (kts, ktin, wks, bkts, S),
                ):
                    for mj in range(DC):
                        for nb in range(ncols // 512):
                            ps = ppsum.tile([128, 512], F32, tag="proj")
                            for ci in range(DC):
                                nc.tensor.matmul(
                                    ps,
                                    lhsT=w[:, ci, mj * 128:(mj + 1) * 128],
                                    rhs=src[:, ci, nb * 512:(nb + 1) * 512],
                                    start=(ci == 0),
                                    stop=(ci == DC - 1),
                                )
                            nc.scalar.activation(
                                out=dst[:, mj, nb * 512:(nb + 1) * 512],
                                in_=ps,
                                func=mybir.ActivationFunctionType.Identity,
                                bias=bias[:, mj:mj + 1],
                            )

                # v natural: out rows = key rows (16 chunks), cols = hdv
                for si in range(KC):
                    for nb in range(2):
                        ps = ppsum.tile([128, 384], F32, tag="proj")
                        for ci in range(DC):
                            nc.tensor.matmul(
                                ps,
                                lhsT=vtin[:, ci, si * 128:(si + 1) * 128],
                                rhs=wvs[:, ci, nb * 384:(nb + 1) * 384],
                                start=(ci == 0),
                                stop=(ci == DC - 1),
                            )
                        nc.vector.tensor_add(
                            vns[:, si, nb * 6:(nb + 1) * 6, 0:DK],
                            ps[:, :].rearrange("p (a b) -> p a b", a=6),
                            bvb[:, nb * 384:(nb + 1) * 384].rearrange(
                                "p (a b) -> p a b", a=6
                            ),
                        )

            # ---------- phase A: attention ----------
            with (
                tc.tile_pool(name="amask", bufs=1) as amask,
                tc.tile_pool(name="anat", bufs=3) as anat,
                tc.tile_pool(name="atr", bufs=3) as atr,
                tc.tile_pool(name="asm", bufs=4) as asm,
                tc.tile_pool(name="afin", bufs=2) as afin,
                tc.tile_pool(name="aout", bufs=2) as aout,
                tc.tile_pool(name="psnat", bufs=1, space="PSUM") as psnat,
                tc.tile_pool(name="pst", bufs=1, space="PSUM") as pst,
                tc.tile_pool(name="psctx", bufs=2, space="PSUM") as psctx,
            ):
                m01s = amask.tile([128, QC, S], BF)
                m01ts = amask.tile([128, KC, NQ], BF)
                nc.sync.dma_start(
                    out=m01s, in_=m01_d.rearrange("(c p) n -> p c n", p=128)
                )
                nc.sync.dma_start(
                    out=m01ts, in_=m01t_d.rearrange("(c p) n -> p c n", p=128)
                )
                for h in range(H):
                    hp = (h % 2) * 64        # partition offset of this head
                    hc = h // 2              # chunk index of this head

                    ctxps = [
                        psctx.tile([DK + 1, 512], F32, tag="ctx", name=f"ctx_{h}_{i}")
                        for i in range(2)
                    ]

                    for step in range(KC):
                        # --- transposed orientation: scores^T [k, q] ---
                        kc = step
                        pst_t = pst.tile([128, NQ], F32, tag="st")
                        for qb in range(NQ // 512):
                            nc.tensor.matmul(
                                pst_t[:, qb * 512:(qb + 1) * 512],
                                lhsT=kts[hp:hp + 64, hc, kc * 128:(kc + 1) * 128],
                                rhs=qts[hp:hp + 64, hc, qb * 512:(qb + 1) * 512],
                                start=True,
                                stop=True,
                            )
                        expt = atr.tile([128, NQ], BF, tag="expt")
                        nc.scalar.activation(
                            out=expt, in_=pst_t,
                            func=mybir.ActivationFunctionType.Exp, scale=SCALE,
                        )
                        expmt = atr.tile([128, NQ], BF, tag="expmt")
                        nc.vector.tensor_mul(expmt, expt, m01ts[:, kc, :])
                        for qb in range(NQ // 512):
                            nc.tensor.matmul(
                                ctxps[qb],
                                lhsT=vns[:, kc, h, :],
                                rhs=expmt[:, qb * 512:(qb + 1) * 512],
                                start=(kc == 0),
                                stop=(kc == KC - 1),
                            )

                        # --- natural orientation: scores [q, k] ---
                        if step < QC:
                            qc = step
                            psn = psnat.tile([128, S], F32, tag="nat")
                            for kb in range(S // 512):
                                nc.tensor.matmul(
                                    psn[:, kb * 512:(kb + 1) * 512],
                                    lhsT=qts[hp:hp + 64, hc, qc * 128:(qc + 1) * 128],
                                    rhs=kts[hp:hp + 64, hc, kb * 512:(kb + 1) * 512],
                                    start=True,
                                    stop=True,
                                )
                            expn = anat.tile([128, S], BF, tag="expn")
                            nc.scalar.activation(
                                out=expn, in_=psn,
                                func=mybir.ActivationFunctionType.Exp, scale=SCALE,
                            )
                            expm = anat.tile([128, S], BF, tag="expm")
                            sums = asm.tile([128, 1], F32, tag="sums")
                            nc.vector.scalar_tensor_tensor(
                                out=expm, in0=expn, scalar=1.0, in1=m01s[:, qc, :],
                                op0=mybir.AluOpType.mult, op1=mybir.AluOpType.mult,
                                accum_out=sums,
                            )
                            rs = asm.tile([128, 1], F32, tag="rs")
                            nc.vector.reciprocal(rs, sums)
                            attn_f = aout.tile([128, S], F32, tag="attn_f")
                            nc.vector.tensor_scalar_mul(attn_f, expm, rs)
                            nc.sync.dma_start(
                                out=attn_o[h, qc * 128:(qc + 1) * 128, :], in_=attn_f
                            )

                    # --- finish head: normalize context^T by the sums row ---
                    for qb in range(2):
                        rrow = afin.tile([1, 512], F32, tag="rrow")
                        nc.vector.reciprocal(rrow, ctxps[qb][DK:DK + 1, :])
                        rbps = pst.tile([DK, 512], F32, tag="st")
                        rrow_bf = afin.tile([1, 512], BF, tag="rrowbf")
                        nc.vector.tensor_copy(rrow_bf, rrow)
                        nc.tensor.matmul(
                            rbps, lhsT=ones, rhs=rrow_bf, start=True, stop=True
                        )
                        rbs = afin.tile([DK, 512], F32, tag="rbs")
                        nc.scalar.copy(rbs, rbps)
                        nc.vector.tensor_mul(
                            ctxs[hp:hp + 64, hc, qb * 512:(qb + 1) * 512],
                            ctxps[qb][0:DK, :],
                            rbs,
                        )

            # ---------- phase O: output projection + residual + LayerNorm ----------
            with (
                tc.tile_pool(name="ow", bufs=1) as ow,
                tc.tile_pool(name="obias", bufs=1) as obias,
                tc.tile_pool(name="orid", bufs=2) as orid,
                tc.tile_pool(name="otmp", bufs=2) as otmp,
                tc.tile_pool(name="osm", bufs=4) as osm,
                tc.tile_pool(name="opsum", bufs=2, space="PSUM") as opsum,
            ):
                wos = ow.tile([128, DC, D], BF)
                nc.sync.dma_start(out=wos, in_=wo_d.rearrange("(c p) n -> p c n", p=128))
                lngb = obias.tile([128, D], F32)
                lnbb = obias.tile([128, D], F32)
                epst = obias.tile([128, 1], F32)
                nc.sync.dma_start(out=lngb, in_=lng_d[:, :].to_broadcast((128, D)))
                nc.sync.dma_start(out=lnbb, in_=lnb_d[:, :].to_broadcast((128, D)))
                nc.vector.memset(epst, float(EPS))

                for qc in range(QC):
                    pss = [
                        opsum.tile([128, 384], F32, tag=f"oproj{i}", name=f"ops_{qc}_{i}")
                        for i in range(2)
                    ]
                    for nb in range(2):
                        for ci in range(DC):
                            nc.tensor.matmul(
                                pss[nb],
                                lhsT=ctxs[:, ci, qc * 128:(qc + 1) * 128],
                                rhs=wos[:, ci, nb * 384:(nb + 1) * 384],
                                start=(ci == 0),
                                stop=(ci == DC - 1),
                            )
                    rtile = orid.tile([128, D], F32, tag="rtile")
                    nc.sync.dma_start(
                        out=rtile,
                        in_=resid_d[qc * 128:(qc + 1) * 128, :],
                    )
                    pre = otmp.tile([128, D], F32, tag="pre")
                    for nb in range(2):
                        nc.vector.tensor_add(
                            pre[:, nb * 384:(nb + 1) * 384],
                            pss[nb],
                            rtile[:, nb * 384:(nb + 1) * 384],
                        )

                    musum = osm.tile([128, 1], F32, tag="musum")
                    nc.vector.tensor_reduce(
                        musum, pre, axis=mybir.AxisListType.X, op=mybir.AluOpType.add
                    )
                    mu = osm.tile([128, 1], F32, tag="mu")
                    nc.scalar.mul(mu, musum, 1.0 / D)
                    cent = otmp.tile([128, D], F32, tag="cent")
                    nc.vector.tensor_scalar(
                        out=cent, in0=pre, scalar1=mu, scalar2=None,
                        op0=mybir.AluOpType.subtract,
                    )
                    sq = otmp.tile([128, D], F32, tag="sq")
                    var = osm.tile([128, 1], F32, tag="var")
                    nc.vector.scalar_tensor_tensor(
                        out=sq, in0=cent, scalar=1.0, in1=cent,
                        op0=mybir.AluOpType.mult, op1=mybir.AluOpType.mult,
                        accum_out=var,
                    )
                    std = osm.tile([128, 1], F32, tag="std")
                    nc.scalar.activation(
                        out=std, in_=var,
                        func=mybir.ActivationFunctionType.Sqrt,
                        scale=1.0 / D, bias=epst[:, 0:1],
                    )
                    rstd = osm.tile([128, 1], F32, tag="rstd")
                    nc.vector.reciprocal(rstd, std)
                    t1 = otmp.tile([128, D], F32, tag="t1")
                    nc.vector.scalar_tensor_tensor(
                        out=t1, in0=cent, scalar=rstd, in1=lngb,
                        op0=mybir.AluOpType.mult, op1=mybir.AluOpType.mult,
                    )
                    outt = otmp.tile([128, D], F32, tag="outt")
                    nc.vector.tensor_add(outt, t1, lnbb)
                    nc.sync.dma_start(
                        out=out_o[qc * 128:(qc + 1) * 128, :], in_=outt
                    )

    if waitsplit:
        _split_multi_waits(nc)
    return nc


_NC = None
TRACE_KWARGS = None   # test harness can set, e.g. {"trace": True, "tmpdir": ...}
LAST_RESULT = None


def _get_nc():
    global _NC
    if _NC is None:
        _NC = _build_nc()
    return _NC


def kernel(Q, K, V, attn_pad, Wq, bq, Wk, bk, Wv, bv, Wo, bo, ln_g, ln_b):
    Q = np.asarray(Q, np.float32)
    K = np.asarray(K, np.float32)
    V = np.asarray(V, np.float32)
    attn_pad = np.asarray(attn_pad, bool)
    Wq = np.asarray(Wq, np.float32)
    Wk = np.asarray(Wk, np.float32)
    Wv = np.asarray(Wv, np.float32)
    Wo = np.asarray(Wo, np.float32)
    bq = np.asarray(bq, np.float32)
    bk = np.asarray(bk, np.float32)
    bv = np.asarray(bv, np.float32)
    bo = np.asarray(bo, np.float32)
    ln_g = np.asarray(ln_g, np.float32)
    ln_b = np.asarray(ln_b, np.float32)

    wq_b = Wq.astype(BF16)
    wk_b = Wk.astype(BF16)
    wv_b = Wv.astype(BF16)
    wo_b = Wo.astype(BF16)
    bqt = np.ascontiguousarray(bq.reshape(DC, 128).T)
    bkt = np.ascontiguousarray(bk.reshape(DC, 128).T)
    bv_row = bv.reshape(1, D)
    lng_row = ln_g.reshape(1, D)
    lnb_row = ln_b.reshape(1, D)

    in_maps = []
    for c in range(8):
        b, qh = c // 2, c % 2
        rows = slice(qh * NQ, (qh + 1) * NQ)
        pad_b = attn_pad[b]
        m01_full = (~pad_b).astype(BF16)          # [S, S]
        in_maps.append({
            "qt": np.ascontiguousarray(Q[b, rows, :].T.astype(BF16)),
            "kt": np.ascontiguousarray(K[b].T.astype(BF16)),
            "vt": np.ascontiguousarray(V[b].T.astype(BF16)),
            "wq": wq_b, "wk": wk_b, "wv": wv_b, "wo": wo_b,
            "bqt": bqt, "bkt": bkt, "bv_row": bv_row,
            "m01": np.ascontiguousarray(m01_full[rows, :]),
            "m01t": np.ascontiguousarray(m01_full[rows, :].T),
            "resid": Q[b, rows, :] + bo[None, :],
            "lng_row": lng_row, "lnb_row": lnb_row,
        })

    nc = _get_nc()
    kwargs = dict(TRACE_KWARGS) if TRACE_KWARGS else {}
    res = run_bass_kernel_spmd(nc, in_maps, core_ids=list(range(8)), **kwargs)
    globals()["LAST_RESULT"] = res

    out = np.empty((B, S, D), np.float32)
    attn = np.empty((B, H, S, S), np.float32)
    for c in range(8):
        b, qh = c // 2, c % 2
        rows = slice(qh * NQ, (qh + 1) * NQ)
        r = res.results[c]
        attn[b, :, rows, :] = r["attn_o"]
        out[b, rows, :] = r["out_o"]
    return out, attn
